# revision 35
# baseline (speedup 1.0000x reference)
"""Trainium2 Bass kernel for cross+self attention (dense_transformer).

Sharding: 8 cores = 2 (batch) x 4 (head-groups of 4 heads).
Each core computes, for its (b, hg):
  qkv projections for its 4 heads (tensor-parallel column split),
  kv projections of the context, rmsnorm(q), rmsnorm(k),
  softmax(q k^T / sqrt(d)) @ v, and a row-split partial of the output
  projection. Host sums the 4 partial proj outputs per batch.

All device tensors keep contraction-on-partitions layouts:
  qT/kT: [d, n] (d on partitions), v: [m, d], S^T: [m, n].
Softmax denominator comes from an appended ones-column in v; logits are
exp'd without max-subtraction (rmsnormed q,k scaled by 1/sqrt(d) give
S ~ N(0,1); fp32 exp overflow is impossible for this distribution).
Activations and weights travel as fp16 (halves DMA bytes and PE weight
load time; fp32 PSUM accumulation keeps precision). Head pairs share
the 128-partition dim; QK^T runs 2 heads concurrently via PE row-tiling
(tile_position from base partitions 0/64).
rmsnorm 1/sqrt on the ACT engine (AF.Rsqrt); softmax reciprocal via the
fast custom-DVE approximation (DVE InstReciprocal is ~4us/call).
"""

import sys
import numpy as np

if '/opt/trn_rl_repo' not in sys.path:
    sys.path.insert(0, '/opt/trn_rl_repo')

import concourse.bacc as bacc
import concourse.mybir as mybir
import concourse.tile as tile
from concourse.bass_utils import run_bass_kernel_spmd
from concourse.dve_ops import RECIP_APPROX_FAST_CONSTS, RECIPROCAL_APPROX_FAST


def _recip_fast(nc, out, in_):
    """1/x at ~51 ULP in one custom-DVE op (~5x faster than the iterative
    InstReciprocal). Output may be f32r/f16 (rounded on write)."""
    c = RECIP_APPROX_FAST_CONSTS
    return nc.vector._custom_dve(
        RECIPROCAL_APPROX_FAST, out=out, in0=in_,
        s0=c["s0"], s1=c["s1"], imm2=c["imm2"])

f32 = mybir.dt.float32
f32r = mybir.dt.float32r
f16 = mybir.dt.float16
AF = mybir.ActivationFunctionType

# problem shapes (hardcoded per contract)
DIM = 1024
HEADS = 16
D = 64
B = 2
N = 2048
M = 512
EPS = 1e-6
SCALE = D ** -0.5  # 0.125

P = 128
CC = DIM // P          # 8 contraction chunks
HG = 4                 # heads per core
NPAIR = 2              # head pairs per core
NB = 512               # n-block width
NNB = N // NB          # 4 n-blocks
MT = N + M             # 2560 total kv length
NMC = MT // P          # 20 m-chunks (0..15 from x, 16..19 from context)
NMCX = N // P          # 16 m-chunks from x

_cached = None


def _build_module():
    nc = bacc.Bacc("TRN2", target_bir_lowering=False, debug=False, num_devices=8)

    xT_d = nc.dram_tensor("xT", [DIM, N], f16, kind="ExternalInput").ap()
    ctxT_d = nc.dram_tensor("ctxT", [DIM, M], f16, kind="ExternalInput").ap()
    wqkvT_d = nc.dram_tensor("wqkvT", [DIM, 3 * HG * D], f16, kind="ExternalInput").ap()
    wkvyT_d = nc.dram_tensor("wkvyT", [DIM, 2 * HG * D], f16, kind="ExternalInput").ap()
    wpT_d = nc.dram_tensor("wpT", [HG * D, DIM], f16, kind="ExternalInput").ap()
    indsum_d = nc.dram_tensor("indsum", [P, 2], f32r, kind="ExternalInput").ap()
    indq_d = nc.dram_tensor("indq", [2, P], f32r, kind="ExternalInput").ap()
    indk_d = nc.dram_tensor("indk", [2, P], f32r, kind="ExternalInput").ap()
    ones64_d = nc.dram_tensor("ones64", [1, D], f32r, kind="ExternalInput").ap()
    out_d = nc.dram_tensor("out", [N, DIM], f32, kind="ExternalOutput").ap()

    with tile.TileContext(nc) as tc:
        _emit(nc, tc, xT_d, ctxT_d, wqkvT_d, wkvyT_d, wpT_d,
              indsum_d, indq_d, indk_d, ones64_d, out_d)
    nc.compile()
    return nc


def _emit(nc, tc, xT_d, ctxT_d, wqkvT_d, wkvyT_d, wpT_d,
          indsum_d, indq_d, indk_d, ones64_d, out_d):
    with (
        tc.tile_pool(name="live", bufs=1) as live,
        tc.tile_pool(name="work", bufs=3) as work,
        tc.tile_pool(name="epool", bufs=6) as epool,
        tc.tile_pool(name="ps", bufs=2, space="PSUM") as ps,
    ):
        # ---- constants / long-lived activations ----
        indsum = live.tile([P, 2], f32r)
        indq = live.tile([2, P], f32r)
        indk = live.tile([2, P], f32r)
        ones64 = live.tile([1, D], f32r)
        eps_t = live.tile([2, 1], f32)
        nc.vector.memset(eps_t[:], EPS)
        qTn = [live.tile([P, N], f16, tag=f"qTn{p}", name=f"qTn{p}") for p in range(NPAIR)]
        kTn = [live.tile([P, MT], f16, tag=f"kTn{p}", name=f"kTn{p}") for p in range(NPAIR)]
        vA = live.tile([P, NMC, HG, D + 1], f16)
        outn = [live.tile([P, N], f16, tag=f"outn{p}", name=f"outn{p}") for p in range(NPAIR)]

        nc.vector.memset(vA[:, :, :, D:D + 1], 1.0)

        # Pipelined emission: each rmsnorm unit emits its matmul chain and
        # square immediately; the dependent finish (sumsq matmul, rsqrt,
        # broadcast matmul, scale) is deferred one unit so the PE stream
        # never waits on DVE/ACT results.
        pend_a = []
        pend_b = []

        def step_finish():
            if pend_b:
                pend_b.pop(0)()
            if pend_a:
                pend_b.append(pend_a.pop(0)())

        def flush_fin():
            while pend_a or pend_b:
                step_finish()

        def norm_chain(w_tile, w_col0, rhs_tile, rhs_slice, width, ind_w,
                       dst, dst_slice):
            """dst[:, dst_slice] = per-head rmsnorm of a [128, width]
            projection chunk (2 heads on partition halves)."""
            acc = ps.tile([P, NB], f32, tag="acc", name="acc")
            for ci in range(CC):
                nc.tensor.matmul(
                    acc[:, :width],
                    w_tile[:, ci, w_col0:w_col0 + P],
                    rhs_tile[:, ci, rhs_slice],
                    start=(ci == 0), stop=(ci == CC - 1))
            raw = work.tile([P, NB], f32r, tag="raw", name="raw")
            nc.vector.tensor_copy(raw[:, :width], acc[:, :width])
            sq = work.tile([P, NB], f32r, tag="sq", name="sq")
            nc.vector.tensor_mul(sq[:, :width], raw[:, :width], raw[:, :width])

            def finish_a():
                ms = ps.tile([2, NB], f32, tag="S", name="ms")
                nc.tensor.matmul(ms[:, :width], indsum[:], sq[:, :width],
                                 start=True, stop=True)
                std = work.tile([2, NB], f32, tag="std", name="std")
                nc.scalar.activation(std[:, :width], ms[:, :width], AF.Sqrt,
                                     scale=1.0 / D, bias=eps_t[:])
                rs = work.tile([2, NB], f32r, tag="rs", name="rs")
                with nc.allow_low_precision(reason="fp22 feeds matmul"):
                    _recip_fast(nc, rs[:, :width], std[:, :width])

                def finish_b():
                    rsb = ps.tile([P, NB], f32, tag="S", name="rsb")
                    nc.tensor.matmul(rsb[:, :width], ind_w[:],
                                     rs[:, :width],
                                     start=True, stop=True)
                    with nc.allow_low_precision(reason="fp16 feeds matmul"):
                        nc.vector.tensor_mul(dst[:, dst_slice],
                                             raw[:, :width], rsb[:, :width])
                return finish_b

            pend_a.append(finish_a)
            step_finish()

        def v_chain(src_tile, src_col0, w_tile, w_col0, mc_global):
            acc = ps.tile([P, NB], f32, tag="acc", name="acc")
            for ci in range(CC):
                nc.tensor.matmul(
                    acc[:, :HG * D],
                    src_tile[:, ci, src_col0:src_col0 + P],
                    w_tile[:, ci, w_col0:w_col0 + HG * D],
                    start=(ci == 0), stop=(ci == CC - 1))
            with nc.allow_low_precision(reason="fp16 feeds matmul"):
                nc.vector.tensor_copy(
                    vA[:, mc_global, :, 0:D],
                    acc[:, :HG * D].rearrange("p (h d) -> p h d", d=D))
            step_finish()

        with tc.tile_pool(name="wq", bufs=1) as wq:
            # context-side weights and activations first: the ky/vy chains
            # are the first consumers, so their DMAs must land first.
            # Split each row-chunk in half so transfers spread over all 16
            # DMA queues.
            wkvy = wq.tile([P, CC, 2 * HG * D], f16)
            wkvy_r = wkvyT_d.rearrange("(o p) c -> p o c", p=P)
            WKW = HG * D
            for ci in range(CC):
                nc.gpsimd.dma_start(wkvy[:, ci, 0:WKW], wkvy_r[:, ci, 0:WKW])
            wqkv = wq.tile([P, CC, 3 * HG * D], f16)
            wqkv_r = wqkvT_d.rearrange("(o p) c -> p o c", p=P)

            # ---- context chains: ky (normed) + vy ----
            with tc.tile_pool(name="ctx", bufs=1) as ctxp:
                ctxT = ctxp.tile([P, CC, M], f16)
                ctxT_r = ctxT_d.rearrange("(o p) m -> p o m", p=P)
                for ci in range(CC):
                    nc.sync.dma_start(ctxT[:, ci, 0:M // 2],
                                      ctxT_r[:, ci, 0:M // 2])
                    nc.scalar.dma_start(ctxT[:, ci, M // 2:M],
                                        ctxT_r[:, ci, M // 2:M])
                # second wkvy halves (vy chains, 3rd consumer) after the
                # ky-critical transfers
                for ci in range(CC):
                    nc.gpsimd.dma_start(wkvy[:, ci, WKW:2 * WKW],
                                        wkvy_r[:, ci, WKW:2 * WKW])
                nc.sync.dma_start(indsum[:], indsum_d[:])
                nc.sync.dma_start(indq[:], indq_d[:])
                nc.sync.dma_start(indk[:], indk_d[:])
                nc.sync.dma_start(ones64[:], ones64_d[:])
                for ci in range(CC):
                    eng = nc.scalar if ci % 2 == 0 else nc.sync
                    eng.dma_start(wqkv[:, ci, :], wqkv_r[:, ci, :])
                norm_chain(wkvy, 0, ctxT, slice(0, M), M, indk,
                           kTn[0], slice(N, N + M))
                v_chain(ctxT, 0, wkvy, HG * D, NMCX + 0)
                v_chain(ctxT, P, wkvy, HG * D, NMCX + 1)
                norm_chain(wkvy, P, ctxT, slice(0, M), M, indk,
                           kTn[1], slice(N, N + M))
                v_chain(ctxT, 2 * P, wkvy, HG * D, NMCX + 2)
                v_chain(ctxT, 3 * P, wkvy, HG * D, NMCX + 3)

            # ---- x chains: kx, q (normed) + vx, streamed per n-block ----
            with tc.tile_pool(name="xp", bufs=2) as xp:
                xT_r = xT_d.rearrange("(o p) n -> p o n", p=P)
                for nb in range(NNB):
                    xq = xp.tile([P, CC, NB], f16, tag="xq")
                    for ci in range(0, CC, 2):
                        nc.sync.dma_start(
                            xq[:, ci:ci + 2, :],
                            xT_r[:, ci:ci + 2, nb * NB:(nb + 1) * NB])
                    # interleave independent v chains between dependent
                    # rmsnorm chains so deferred finishes never stall PE
                    norm_chain(wqkv, HG * D, xq, slice(0, NB),
                               NB, indk, kTn[0],
                               slice(nb * NB, (nb + 1) * NB))
                    v_chain(xq, 0, wqkv, 2 * HG * D, nb * (NB // P) + 0)
                    norm_chain(wqkv, HG * D + P, xq, slice(0, NB),
                               NB, indk, kTn[1],
                               slice(nb * NB, (nb + 1) * NB))
                    v_chain(xq, P, wqkv, 2 * HG * D, nb * (NB // P) + 1)
                    norm_chain(wqkv, 0, xq, slice(0, NB), NB, indq,
                               qTn[0], slice(nb * NB, (nb + 1) * NB))
                    v_chain(xq, 2 * P, wqkv, 2 * HG * D, nb * (NB // P) + 2)
                    norm_chain(wqkv, P, xq, slice(0, NB), NB, indq,
                               qTn[1], slice(nb * NB, (nb + 1) * NB))
                    v_chain(xq, 3 * P, wqkv, 2 * HG * D, nb * (NB // P) + 3)

        flush_fin()

        wpT = live.tile([P, NPAIR, DIM], f16)
        wpT_r = wpT_d.rearrange("(o p) c -> p o c", p=P)
        for pr in range(NPAIR):
            nc.gpsimd.dma_start(wpT[:, pr, :], wpT_r[:, pr, :])

        # ---- attention + proj, per n-block ----
        # AV matmuls are emitted with a 2-slot lag so the PE instruction
        # stream never head-of-line blocks on an exp; normalize and proj
        # are deferred through a FIFO flushed from slot 2 onward (after
        # the lagged AVs of the previous iteration have been emitted).
        from collections import deque
        pend_av = deque()
        tail_q = deque()

        def emit_av(item):
            av_et, av_ot, p_, mc_, e_tt, fins = item
            nc.tensor.matmul(av_et[:], vA[:, mc_, 2 * p_, :], e_tt[:, 0, :],
                             start=(mc_ == 0), stop=(mc_ == NMC - 1))
            nc.tensor.matmul(av_ot[:], vA[:, mc_, 2 * p_ + 1, :],
                             e_tt[:, 1, :],
                             start=(mc_ == 0), stop=(mc_ == NMC - 1))
            if fins is not None:
                for f in fins:
                    f()

        def make_norm_parts(p, nb, which, av_t):
            nbs = slice(nb * NB, (nb + 1) * NB)
            box = {}

            def _norm_dve():
                # custom-DVE ops misread PSUM at partition offsets; bounce
                # the denominator row through SBUF first.
                den = work.tile([1, NB], f32, tag="den", name="den")
                nc.vector.tensor_copy(den[:], av_t[D:D + 1, :])
                rc1 = work.tile([1, NB], f32r, tag="rc1", name="rc1")
                with nc.allow_low_precision(reason="fp22 feeds matmul"):
                    _recip_fast(nc, rc1[:], den[:])
                avn = work.tile([D, NB], f32, tag="avn", name="avn")
                nc.vector.tensor_copy(avn[:], av_t[0:D, :])
                box['rc1'] = rc1
                box['avn'] = avn

            def _norm_pe():
                rcb = ps.tile([D, NB], f32, tag="acc", name="rcb")
                nc.tensor.matmul(rcb[:], ones64[:], box['rc1'][:],
                                 start=True, stop=True)
                with nc.allow_low_precision(reason="fp16 feeds matmul"):
                    nc.vector.tensor_mul(
                        outn[p][which * D:(which + 1) * D, nbs],
                        box['avn'][:], rcb[:])
            return _norm_dve, _norm_pe

        def make_proj_unit(nb, nch, co):
            def _proj():
                n0 = nb * NB + nch * P
                pp = ps.tile([P, NB], f32, tag="acc", name="pp")
                for pr in range(NPAIR):
                    nc.tensor.matmul(pp[:], outn[pr][:, n0:n0 + P],
                                     wpT[:, pr, co * NB:(co + 1) * NB],
                                     start=(pr == 0),
                                     stop=(pr == NPAIR - 1))
                po = work.tile([P, NB], f32, tag="po", name="po")
                nc.vector.tensor_copy(po[:], pp[:])
                c0 = co * NB
                nc.sync.dma_start(
                    out_d[n0:n0 + P, c0:c0 + NB // 2], po[:, 0:NB // 2])
                nc.gpsimd.dma_start(
                    out_d[n0:n0 + P, c0 + NB // 2:c0 + NB],
                    po[:, NB // 2:NB])
            return _proj

        for nb in range(NNB):
            nbs = slice(nb * NB, (nb + 1) * NB)
            for p in range(NPAIR):
                av_e = ps.tile([D + 1, NB], f32, tag="av", name="av_e")
                av_o = ps.tile([D + 1, NB], f32, tag="av", name="av_o")
                ae_dve, ae_pe = make_norm_parts(p, nb, 0, av_e)
                ao_dve, ao_pe = make_norm_parts(p, nb, 1, av_o)
                for mc in range(NMC):
                    s_t = ps.tile([P, 2, NB], f32, tag="S", name="s_t")
                    nc.tensor.matmul(s_t[:, 0, :],
                                     kTn[p][0:D, mc * P:(mc + 1) * P],
                                     qTn[p][0:D, nbs], start=True, stop=True)
                    nc.tensor.matmul(s_t[:, 1, :],
                                     kTn[p][D:P, mc * P:(mc + 1) * P],
                                     qTn[p][D:P, nbs], start=True, stop=True)
                    e_t = epool.tile([P, 2, NB], f16, tag="E", name="e_t")
                    nc.scalar.activation(
                        e_t[:].rearrange("p a b -> p (a b)"),
                        s_t[:].rearrange("p a b -> p (a b)"),
                        AF.Exp, scale=SCALE)
                    if 4 <= mc <= NMC - 4 and tail_q:
                        tail_q.popleft()()
                    fins = ((ae_dve, ao_dve) if mc == NMC - 1 else None)
                    pend_av.append((av_e, av_o, p, mc, e_t, fins))
                    if len(pend_av) > 4:
                        emit_av(pend_av.popleft())
                tail_q.append(ae_pe)
                tail_q.append(ao_pe)
            for nch in range(NB // P):
                for co in range(2):
                    tail_q.append(make_proj_unit(nb, nch, co))
        while pend_av:
            emit_av(pend_av.popleft())
        while tail_q:
            tail_q.popleft()()


def _get_module():
    global _cached
    if _cached is None:
        _cached = _build_module()
    return _cached


def _make_in_maps(x, context, qkv_w, kv_y_w, proj_w, q_norm_w, k_norm_w):
    GD = HG * D  # 256 head-dims per core
    indsum = np.zeros((P, 2), np.float32)
    indsum[0:D, 0] = 1.0
    indsum[D:P, 1] = 1.0
    indq = np.zeros((2, P), np.float32)
    indq[0, 0:D] = q_norm_w
    indq[1, D:P] = q_norm_w
    indk = np.zeros((2, P), np.float32)
    indk[0, 0:D] = k_norm_w
    indk[1, D:P] = k_norm_w
    ones64 = np.ones((1, D), np.float32)
    projT = np.ascontiguousarray(proj_w.T)  # [ci, co]

    in_maps = []
    for core in range(8):
        b, hg = divmod(core, 4)
        r0 = hg * GD
        wq = qkv_w[r0:r0 + GD]
        wk = qkv_w[DIM + r0:DIM + r0 + GD]
        wv = qkv_w[2 * DIM + r0:2 * DIM + r0 + GD]
        wky = kv_y_w[r0:r0 + GD]
        wvy = kv_y_w[DIM + r0:DIM + r0 + GD]
        in_maps.append({
            "xT": np.ascontiguousarray(x[b].T.astype(np.float16)),
            "ctxT": np.ascontiguousarray(context[b].T.astype(np.float16)),
            "wqkvT": np.ascontiguousarray(
                np.concatenate([wq, wk, wv], 0).T.astype(np.float16)),
            "wkvyT": np.ascontiguousarray(
                np.concatenate([wky, wvy], 0).T.astype(np.float16)),
            "wpT": np.ascontiguousarray(projT[r0:r0 + GD].astype(np.float16)),
            "indsum": indsum,
            "indq": indq,
            "indk": indk,
            "ones64": ones64,
        })
    return in_maps


def kernel(x, context, qkv_w, kv_y_w, proj_w, proj_b, q_norm_w, k_norm_w):
    x = np.asarray(x, np.float32)
    context = np.asarray(context, np.float32)
    qkv_w = np.asarray(qkv_w, np.float32)
    kv_y_w = np.asarray(kv_y_w, np.float32)
    proj_w = np.asarray(proj_w, np.float32)
    proj_b = np.asarray(proj_b, np.float32)
    q_norm_w = np.asarray(q_norm_w, np.float32)
    k_norm_w = np.asarray(k_norm_w, np.float32)

    nc = _get_module()
    in_maps = _make_in_maps(x, context, qkv_w, kv_y_w, proj_w,
                            q_norm_w, k_norm_w)
    res = run_bass_kernel_spmd(nc, in_maps, core_ids=list(range(8)))
    out = np.zeros((B, N, DIM), np.float32)
    for core in range(8):
        b = core // 4
        out[b] += res.results[core]["out"]
    out += proj_b[None, None, :]
    return out
